# revision 15
# baseline (speedup 1.0000x reference)
"""Performer (FAVOR+) attention kernel for Trainium2, 8 NeuronCores.

Problem: T=8 tasks, N=M=1024 seq, H=8 heads, D=E=256, NB=1419 random features.
Sharding: data-parallel over tasks (one task per core, zero communication).

Per-core math (task t, head h), with ratio dropped (cancels in num/den):
  qa = (q*nrm) @ Wq[h] + bq[h]*nrm          (computed transposed: qaT [e,n])
  kd[n,m] = qaT/kaT' dash = ka_s @ projT     (psum [n,m])
  G_k = exp(kd - dg_k[n])  (bf16, [n,m])     rsGk[n] = rowsum (ACT accum)
  beta = 1/max_nm(G_k * e^{dg_k}) = e^{-stab_k}
  vp = v @ (Wv@Wo_h^T) + bv@Wo_h^T           ([n, d] with ones col 256)
  CGp[m, 0:257] = sum_n G_k[n,m] * vp_aug[n,:]   (psum, per m-slice)
  Cp = beta*CGp + eps*[vps | 1024]           (bf16, [m, 257])
  G_q = exp(qd - dg_q[n]) (bf16 [n,m]) -> DMA-transpose -> GqT [m,n]
  w[n] = eps * rowmax(G_q) * e^{dg_q[n]}     (= eps * e^{stab_q[n]})
  num[n, 0:257] = sum_m GqT[m,n] * Cp[m,:]   (psum)
  den = num[:,256] + w*kappa ; rep_h = (num[:,0:256] + w*csp)/den
  rep = sum_h rep_h + b_out
"""

import math
import sys

import numpy as np

sys.path.insert(0, "/opt/trn_rl_repo")

import concourse.bass as bass  # noqa: E402
import concourse.bacc as bacc_mod  # noqa: E402
import concourse.mybir as mybir  # noqa: E402
import concourse.tile as tile  # noqa: E402
from concourse.bass_utils import run_bass_kernel_spmd  # noqa: E402

T, N, H, D = 8, 1024, 8, 256
NB = 1419  # int(D * log(D))
MT = 12  # m tiles of 128 (padded region 1419:1536 handled explicitly)
NS = 8  # n slices of 128
EPS = 1e-4
F32 = mybir.dt.float32
F32R = mybir.dt.float32r
BF16 = mybir.dt.bfloat16
AX = mybir.AxisListType
OP = mybir.AluOpType
AF = mybir.ActivationFunctionType

USE_GPSIMD_MAX = False  # InstMax is DVE anyway; use reduce_max

_NC_CACHE = {}


def build_program():
    nc = bacc_mod.Bacc()

    qt_d = nc.declare_dram_parameter("qt", [D, N], F32R, isOutput=False)
    kt_d = nc.declare_dram_parameter("kt", [D, N], F32R, isOutput=False)
    vt_d = nc.declare_dram_parameter("vt", [D, N], F32R, isOutput=False)
    wq_d = nc.declare_dram_parameter("wq", [H, D, D], F32R, isOutput=False)
    wk_d = nc.declare_dram_parameter("wk", [H, D, D], F32R, isOutput=False)
    bqs_d = nc.declare_dram_parameter("bqs", [H, D], F32R, isOutput=False)
    bks_d = nc.declare_dram_parameter("bks", [H, D], F32R, isOutput=False)
    wvo_d = nc.declare_dram_parameter("wvo", [H, D + 1, D + 2], F32R, isOutput=False)
    projt_d = nc.declare_dram_parameter("projt", [D, NB + 1], F32R, isOutput=False)
    bout_d = nc.declare_dram_parameter("bout", [D], F32, isOutput=False)
    ones_d = nc.declare_dram_parameter("onesv", [N], F32R, isOutput=False)
    out_d = nc.declare_dram_parameter("out", [N, D], F32, isOutput=True)

    with tile.TileContext(nc) as tc:
        _build_tile(nc, tc, qt_d, kt_d, vt_d, wq_d, wk_d, bqs_d, bks_d,
                    wvo_d, projt_d, bout_d, out_d, ones_d)
    nc.finalize()
    return nc


def _build_tile(nc, tc, qt_d, kt_d, vt_d, wq_d, wk_d, bqs_d, bks_d,
                wvo_d, projt_d, bout_d, out_d, ones_d):
    from contextlib import ExitStack
    ctx = ExitStack()
    with ctx:
        singles = ctx.enter_context(tc.tile_pool(name="singles", bufs=1))
        wpool = ctx.enter_context(tc.tile_pool(name="wpool", bufs=2))
        hpool = ctx.enter_context(tc.tile_pool(name="hpool", bufs=2))
        gpool = ctx.enter_context(tc.tile_pool(name="gpool", bufs=1))
        spool = ctx.enter_context(tc.tile_pool(name="spool", bufs=2))
        sqpool = ctx.enter_context(tc.tile_pool(name="sqpool", bufs=1))
        psbig = ctx.enter_context(tc.tile_pool(name="psbig", bufs=2, space="PSUM"))
        pssm = ctx.enter_context(tc.tile_pool(name="pssm", bufs=2, space="PSUM"))

        # ---- persistent loads ----
        qt_sb = singles.tile([128, 2, N], F32R, tag="qt")
        kt_sb = singles.tile([128, 2, N], F32R, tag="kt")
        vt_sb = singles.tile([128, 2, N], F32R, tag="vt")
        nc.sync.dma_start(out=qt_sb, in_=qt_d.rearrange("(o p) n -> p o n", p=128))
        nc.sync.dma_start(out=kt_sb, in_=kt_d.rearrange("(o p) n -> p o n", p=128))
        nc.sync.dma_start(out=vt_sb, in_=vt_d.rearrange("(o p) n -> p o n", p=128))
        projt_sb = singles.tile([128, 2, NB + 1], F32R, tag="projt")
        nc.sync.dma_start(out=projt_sb, in_=projt_d.rearrange("(o p) m -> p o m", p=128))
        bout_bc = singles.tile([128, D], F32, tag="bout")
        nc.sync.dma_start(out=bout_bc, in_=bout_d[None, :].to_broadcast((128, D)))
        ones_row = singles.tile([1, N], F32R, tag="ones_row")
        nc.sync.dma_start(out=ones_row, in_=ones_d[None, :])
        ones_col = singles.tile([128, 1], F32, tag="ones_col")
        nc.vector.memset(ones_col, 1.0)
        ones_brow = singles.tile([1, 128], F32, tag="ones_brow")
        nc.vector.memset(ones_brow, 1.0)

        # persistent accumulators
        rep_acc = singles.tile([128, NS, D], F32, tag="rep_acc")

        # m-chunks for dash matmuls (free dim)
        mchunks = [(0, 512), (512, 512), (1024, NB + 1 - 1024)]

        import os
        NH = int(os.environ.get("KERNEL_NHEADS", str(H)))
        for h in range(NH):
            # ---- per-head weight loads ----
            wq_sb = wpool.tile([128, 2, D], F32R, tag="wq")
            wk_sb = wpool.tile([128, 2, D], F32R, tag="wk")
            nc.sync.dma_start(out=wq_sb, in_=wq_d[h].rearrange("(o p) e -> p o e", p=128))
            nc.sync.dma_start(out=wk_sb, in_=wk_d[h].rearrange("(o p) e -> p o e", p=128))
            bq_row = wpool.tile([1, D], F32R, tag="bq")
            bk_row = wpool.tile([1, D], F32R, tag="bk")
            nc.sync.dma_start(out=bq_row, in_=bqs_d[h][None, :])
            nc.sync.dma_start(out=bk_row, in_=bks_d[h][None, :])
            wvo_sb = wpool.tile([128, 2, D + 2], F32R, tag="wvo")
            nc.sync.dma_start(out=wvo_sb,
                              in_=wvo_d[h, 0:D].rearrange("(o p) e -> p o e", p=128))
            wvo_r = wpool.tile([1, D + 2], F32R, tag="wvor")
            nc.sync.dma_start(out=wvo_r, in_=wvo_d[h, D][None, :])

            # ---- projections: qaT/kaT [e,n] = W^T x + b ----
            qaT = hpool.tile([128, 2, N], F32R, tag="qaT")
            kaT = hpool.tile([128, 2, N], F32R, tag="kaT")
            sq_q = sqpool.tile([128, 2, N], F32, tag="sq_q")
            sq_k = sqpool.tile([128, 2, N], F32, tag="sq_k")
            sq_map = {id(qaT): sq_q, id(kaT): sq_k}
            for (dst, w_sb, b_row, src) in ((qaT, wq_sb, bq_row, qt_sb),
                                            (kaT, wk_sb, bk_row, kt_sb)):
                for et in range(2):
                    ps = psbig.tile([128, N], F32, tag="big")
                    esl = slice(et * 128, (et + 1) * 128)
                    for nk in range(2):
                        nsl = slice(nk * 512, (nk + 1) * 512)
                        for dk in range(2):
                            nc.tensor.matmul(ps[:, nsl], w_sb[:, dk, esl],
                                             src[:, dk, nsl],
                                             start=(dk == 0), stop=False)
                        nc.tensor.matmul(ps[:, nsl], b_row[0:1, esl],
                                         ones_row[0:1, nsl],
                                         start=False, stop=True)
                    nc.scalar.copy(out=dst[:, et, :], in_=ps)
                    nc.scalar.activation(out=sq_map[id(dst)][:, et, :], in_=ps,
                                         func=AF.Square)

            # ---- diag: dg = 0.5*sum_e aT^2  -> neg_dg [128, NS] ----
            negdg = spool.tile([128, NS], F32, tag="negdg")
            negdk = spool.tile([128, NS], F32, tag="negdk")
            edg = spool.tile([128, NS], F32, tag="edg")
            edk = spool.tile([128, NS], F32, tag="edk")
            for (sq, negd, ed) in ((sq_q, negdg, edg), (sq_k, negdk, edk)):
                dps = pssm.tile([128, D + 2], F32, tag="small")
                for ns in range(NS):
                    for et in range(2):
                        nc.tensor.matmul(
                            dps[:, ns:ns + 1],
                            sq[:, et, ns * 128:(ns + 1) * 128],
                            ones_col,
                            start=(et == 0), stop=(et == 1))
                nc.vector.tensor_scalar_mul(negd, dps[:, 0:NS], -0.5)
                # e^{+dg} = exp(-1 * negd)
                nc.scalar.activation(out=ed, in_=negd, func=AF.Exp, scale=-1.0)

            # ---- vp [n, 257] = v @ Wvo + bvo (col 256 = ones) ----
            vpa = hpool.tile([128, NS, D + 1], BF16, tag="vpa")
            for ns in range(NS):
                vps_ = pssm.tile([128, D + 2], F32, tag="small")
                nsl = slice(ns * 128, (ns + 1) * 128)
                for dk in range(2):
                    nc.tensor.matmul(vps_[:, 0:D + 2], vt_sb[:, dk, nsl],
                                     wvo_sb[:, dk, :],
                                     start=(dk == 0), stop=False)
                nc.tensor.matmul(vps_[:, 0:D + 2], ones_row[0:1, 0:128],
                                 wvo_r, start=False, stop=True)
                nc.scalar.copy(out=vpa[:, ns, :], in_=vps_[:, 0:D + 1])

            # ---- k side: kd -> G_k (bf16), rsGk, rowmax ----
            Gk = gpool.tile([128, NS, 1536], BF16, tag="Gk")
            rsGk = spool.tile([128, NS], F32, tag="rsGk")
            rmk = spool.tile([128, NS], F32, tag="rmk")
            for ns in range(NS):
                kd = psbig.tile([128, NB + 1], F32, tag="big")
                nsl = slice(ns * 128, (ns + 1) * 128)
                for (mof, msz) in mchunks:
                    for ek in range(2):
                        nc.tensor.matmul(kd[:, mof:mof + msz],
                                         kaT[:, ek, nsl],
                                         projt_sb[:, ek, mof:mof + msz],
                                         start=(ek == 0), stop=(ek == 1))
                nc.scalar.activation(out=Gk[:, ns, 0:NB], in_=kd[:, 0:NB],
                                     func=AF.Exp, bias=negdk[:, ns:ns + 1],
                                     scale=1.0, accum_out=rsGk[:, ns:ns + 1])
                nc.vector.memset(Gk[:, ns, NB:1536], 0.0)
                nc.vector.memset(Gk[:, ns, NB:NB + 1], 1.0)
                nc.vector.reduce_max(out=rmk[:, ns:ns + 1],
                                     in_=Gk[:, ns, 0:NB], axis=AX.X)

            # ---- stab_k -> beta = 1/max_n(rmk * e^{dg_k}) ----
            tk = spool.tile([128, NS], F32, tag="tk")
            nc.vector.tensor_tensor(out=tk, in0=rmk, in1=edk, op=OP.mult)
            colmax = spool.tile([128, 1], F32, tag="colmax")
            nc.vector.reduce_max(out=colmax, in_=tk, axis=AX.X)
            row128 = spool.tile([1, 128], F32, tag="row128")
            nc.sync.dma_start(out=row128, in_=colmax)
            mstar = spool.tile([1, 1], F32, tag="mstar")
            nc.vector.reduce_max(out=mstar, in_=row128, axis=AX.X)
            beta11 = spool.tile([1, 1], F32, tag="beta11")
            nc.vector.reciprocal(out=beta11, in_=mstar)
            bps = pssm.tile([128, D + 2], F32, tag="small")
            nc.tensor.matmul(bps[:, 0:1], ones_brow, beta11,
                             start=True, stop=True)
            beta_bc = spool.tile([128, 1], F32, tag="beta_bc")
            nc.vector.tensor_copy(out=beta_bc, in_=bps[:, 0:1])

            # ---- q side: qd -> G_q (bf16) -> rowmax + transpose ----
            GqT = gpool.tile([128, MT, N], BF16, tag="GqT")
            rmq = spool.tile([128, NS], F32, tag="rmq")
            for ns in range(NS):
                qd = psbig.tile([128, NB + 1], F32, tag="big")
                nsl = slice(ns * 128, (ns + 1) * 128)
                for (mof, msz) in mchunks:
                    for ek in range(2):
                        nc.tensor.matmul(qd[:, mof:mof + msz],
                                         qaT[:, ek, nsl],
                                         projt_sb[:, ek, mof:mof + msz],
                                         start=(ek == 0), stop=(ek == 1))
                Gq = hpool.tile([128, 1536], BF16, tag="Gq")
                nc.scalar.activation(out=Gq[:, 0:NB], in_=qd[:, 0:NB],
                                     func=AF.Exp, bias=negdg[:, ns:ns + 1],
                                     scale=1.0)
                nc.vector.memset(Gq[:, NB:1536], 0.0)
                nc.vector.reduce_max(out=rmq[:, ns:ns + 1],
                                     in_=Gq[:, 0:NB], axis=AX.X)
                nc.sync.dma_start_transpose(GqT[:, :, nsl], Gq)

            # w = eps * rmq * e^{dg_q}
            w_sb = spool.tile([128, NS], F32, tag="w_sb")
            nc.vector.tensor_tensor(out=w_sb, in0=rmq, in1=edg, op=OP.mult)
            nc.vector.tensor_scalar_mul(w_sb, w_sb, EPS)

            # ---- CGp: context in d space; ms=11 first (vps row) ----
            Cpa = hpool.tile([128, MT, D + 1], BF16, tag="Cpa")
            corr = spool.tile([1, D + 2], F32, tag="corr")
            corr2 = spool.tile([1, D + 2], F32, tag="corr2")
            for ms in [MT - 1] + list(range(MT - 1)):
                cg = pssm.tile([128, D + 2], F32, tag="small")
                for ns in range(NS):
                    nc.tensor.matmul(cg[:, 0:D + 1],
                                     Gk[:, ns, ms * 128:(ms + 1) * 128],
                                     vpa[:, ns, :],
                                     start=(ns == 0), stop=(ns == NS - 1))
                if ms == MT - 1:
                    # partition 11 row = [vps | 1024]; eps-corr rows
                    cg11_sb = spool.tile([128, D + 1], F32, tag="cg11_sb")
                    nc.scalar.copy(out=cg11_sb, in_=cg[:, 0:D + 1])
                    vps_sb = spool.tile([1, D + 2], F32, tag="vps_sb")
                    nc.vector.memset(vps_sb, 0.0)
                    nc.sync.dma_start(out=vps_sb[0:1, 0:D + 1],
                                      in_=cg11_sb[11:12, :])
                    nc.vector.tensor_scalar_mul(corr, vps_sb, EPS)
                    nc.vector.tensor_scalar_mul(corr2, corr, float(NB))
                    cps_ = pssm.tile([128, D + 2], F32, tag="small")
                    nc.tensor.matmul(cps_[:, 0:D + 2], ones_brow, corr,
                                     start=True, stop=True)
                    corr_bc = spool.tile([128, D + 1], F32, tag="corr_bc")
                    nc.vector.tensor_copy(out=corr_bc, in_=cps_[:, 0:D + 1])
                    nc.vector.memset(Cpa[:, ms, :], 0.0)
                    nc.vector.scalar_tensor_tensor(
                        out=Cpa[0:11, ms, :], in0=cg11_sb[0:11],
                        scalar=beta_bc[0:11], in1=corr_bc[0:11],
                        op0=OP.mult, op1=OP.add)
                else:
                    nc.vector.scalar_tensor_tensor(
                        out=Cpa[:, ms, :], in0=cg[:, 0:D + 1],
                        scalar=beta_bc, in1=corr_bc,
                        op0=OP.mult, op1=OP.add)

            # ---- csp row: column sums of Cp ----
            rsGk_bf = spool.tile([128, NS], BF16, tag="rsGk_bf")
            nc.vector.tensor_copy(out=rsGk_bf, in_=rsGk)
            cs = pssm.tile([128, D + 2], F32, tag="small")
            for ns in range(NS):
                nc.tensor.matmul(cs[0:1, 0:D + 1], rsGk_bf[:, ns:ns + 1],
                                 vpa[:, ns, :],
                                 start=(ns == 0), stop=(ns == NS - 1))
            csp = spool.tile([1, D + 2], F32, tag="csp")
            nc.vector.memset(csp, 0.0)
            nc.vector.scalar_tensor_tensor(
                out=csp[0:1, 0:D + 1], in0=cs[0:1, 0:D + 1], scalar=beta11,
                in1=corr2[0:1, 0:D + 1], op0=OP.mult, op1=OP.add)
            csps_ = pssm.tile([128, D + 2], F32, tag="small")
            nc.tensor.matmul(csps_[:, 0:D + 2], ones_brow, csp,
                             start=True, stop=True)
            csp_bc = spool.tile([128, D + 1], F32, tag="csp_bc")
            nc.vector.tensor_copy(out=csp_bc, in_=csps_[:, 0:D + 1])

            # ---- numerator + rep accumulation ----
            for ns in range(NS):
                nm = pssm.tile([128, D + 2], F32, tag="small")
                nsl = slice(ns * 128, (ns + 1) * 128)
                for ms in range(MT):
                    nc.tensor.matmul(nm[:, 0:D + 1], GqT[:, ms, nsl],
                                     Cpa[:, ms, :],
                                     start=(ms == 0), stop=(ms == MT - 1))
                den = spool.tile([128, 1], F32, tag="den")
                nc.vector.scalar_tensor_tensor(
                    out=den, in0=w_sb[:, ns:ns + 1], scalar=csp_bc[:, D:D + 1],
                    in1=nm[:, D:D + 1], op0=OP.mult, op1=OP.add)
                dinv = spool.tile([128, 1], F32, tag="dinv")
                nc.vector.reciprocal(out=dinv, in_=den)
                wd = spool.tile([128, 1], F32, tag="wd")
                nc.vector.tensor_tensor(out=wd, in0=w_sb[:, ns:ns + 1],
                                        in1=dinv, op=OP.mult)
                if h == 0:
                    in1_first = bout_bc
                else:
                    in1_first = rep_acc[:, ns, :]
                nc.vector.scalar_tensor_tensor(
                    out=rep_acc[:, ns, :], in0=csp_bc[:, 0:D],
                    scalar=wd, in1=in1_first, op0=OP.mult, op1=OP.add)
                nc.vector.scalar_tensor_tensor(
                    out=rep_acc[:, ns, :], in0=nm[:, 0:D], scalar=dinv,
                    in1=rep_acc[:, ns, :], op0=OP.mult, op1=OP.add)
                if h == NH - 1:
                    nc.sync.dma_start(out=out_d[ns * 128:(ns + 1) * 128, :],
                                      in_=rep_acc[:, ns, :])


def kernel(**inputs):
    q = np.asarray(inputs["q"], np.float32)
    k = np.asarray(inputs["k"], np.float32)
    v = np.asarray(inputs["v"], np.float32)
    Wq = np.asarray(inputs["Wq"], np.float32)
    bq = np.asarray(inputs["bq"], np.float32)
    Wk = np.asarray(inputs["Wk"], np.float32)
    bk = np.asarray(inputs["bk"], np.float32)
    Wv = np.asarray(inputs["Wv"], np.float32)
    bv = np.asarray(inputs["bv"], np.float32)
    W_out = np.asarray(inputs["W_out"], np.float32)
    b_out = np.asarray(inputs["b_out"], np.float32)
    proj = np.asarray(inputs["proj"], np.float32)

    nrm = float(D) ** -0.25
    Wo = W_out.reshape(D, D, H)  # [d_out, e, h]
    wvo = np.zeros((H, D + 1, D + 2), np.float32)
    for h in range(H):
        wvo[h, 0:D, 0:D] = Wv[h] @ Wo[:, :, h].T  # [din, dout]
        wvo[h, D, 0:D] = bv[h] @ Wo[:, :, h].T
        wvo[h, D, D] = 1.0
    projt = np.zeros((D, NB + 1), np.float32)  # [256, 1420], last col pad
    projt[:, 0:NB] = proj.T

    shared = {
        "wq": np.ascontiguousarray(Wq), "wk": np.ascontiguousarray(Wk),
        "bqs": np.ascontiguousarray(bq * nrm), "bks": np.ascontiguousarray(bk * nrm),
        "wvo": wvo, "projt": projt,
        "bout": np.ascontiguousarray(b_out),
        "onesv": np.ones((N,), np.float32),
    }
    in_maps = []
    for t in range(T):
        m = dict(shared)
        m["qt"] = np.ascontiguousarray(q[t].T * nrm)
        m["kt"] = np.ascontiguousarray(k[t].T * nrm)
        m["vt"] = np.ascontiguousarray(v[t].T)
        in_maps.append(m)

    if "nc" not in _NC_CACHE:
        _NC_CACHE["nc"] = build_program()
    nc = _NC_CACHE["nc"]
    res = run_bass_kernel_spmd(nc, in_maps, list(range(T)))
    out = np.stack([np.asarray(res.results[i]["out"]) for i in range(T)])
    return out.astype(np.float32)


if __name__ == "__main__":
    np.random.seed(0)
    ins = {
        "q": np.random.randn(T, N, D).astype(np.float32),
        "k": np.random.randn(T, N, D).astype(np.float32),
        "v": np.random.randn(T, N, D).astype(np.float32),
        "Wq": np.random.randn(H, D, D).astype(np.float32) / 16,
        "bq": np.random.randn(H, D).astype(np.float32) * 0.01,
        "Wk": np.random.randn(H, D, D).astype(np.float32) / 16,
        "bk": np.random.randn(H, D).astype(np.float32) * 0.01,
        "Wv": np.random.randn(H, D, D).astype(np.float32) / 16,
        "bv": np.random.randn(H, D).astype(np.float32) * 0.01,
        "W_out": np.random.randn(D, H * D).astype(np.float32) / 45,
        "b_out": np.random.randn(D).astype(np.float32) * 0.01,
        "proj": np.random.randn(NB, D).astype(np.float32),
    }
    out = kernel(**ins)
    print(out.shape, out.dtype)
